# revision 39
# baseline (speedup 1.0000x reference)
"""Trainium2 Bass kernel for nn_Attention_51608327028778 (sparse_attention).

Problem (hardcoded shapes):
  T=32, N=16, V=64, C=128, mT=32, mV=64
  P:[32,1024,128] M:[32,1024,128] mask:[16,1,64,1] Wq/Wk/Wv:[128,128] b*:[128]
  out:[32,1024,128], att:[16,64,64]

Key ideas:
 - the mask is per-(scene, mv) and shared across mT, so the mv axis is
   COMPACTED host-side to the valid entries (padded to a per-section NVP),
   shrinking scores / softmax / AV / att work by ~ NVP/mV.  Keys are ordered
   k' = mt*NVP + c (c = compact mv index) so softmax-over-mv becomes
   partition-block sums.
 - scenes are SORTED by valid count and paired big-with-big across cores:
   graph section 0 (first scene on every core) gets the 8 largest scenes,
   section 1 the 8 smallest, so each section pads to its own (smaller) NVP.
 - schedule: PE warm-up burst; then all softmax units (scores/exp/Z/recip/
   normalize, chains hidden under the next unit's scores); then one dense
   all-matmul output block.

Per-scene math (scene n):
  Q = Pp@Wq.T+bq ; K = Mpc@Wk.T+bk ; Val = Mpc@Wv.T (+bv folded into output)
  S^T[k',q] = K@Q^T                    (fp16 matmuls, 1 cyc/row)
  A = exp(S^T + padbias[k'])           (ACT, per-partition bias, bf16 out)
  Z[mt,q] += G2_kt^T @ A_kt            (TensorE; G2 structural)
  rz = reciprocal_approx_fast(Z); replicated across partitions via DRAM
  a^T = A * rz_rep                     (DVE bf16 2x)
  out^T[c,q] += Val_kt^T a^T           (TensorE, accum over k' tiles)
  att^T[mv,q] += (H_n/1024)^T a^T      (TensorE), then reduce over t (DVE)
  out^T[c,q] += mT*bv[c]               (ACT bias on PSUM eviction; exact
                                        because softmax sums to 1 per (q,mt))

Sharding: data-parallel over scenes, 2 scenes per core, 8 cores, full I/O
resharded host-side.
"""

import os
import sys

import numpy as np

for _p in ("/opt/trn_rl_repo",):
    if _p not in sys.path and os.path.isdir(_p):
        sys.path.append(_p)

import ml_dtypes  # noqa: E402

import concourse.tile as tile  # noqa: E402
from concourse import bacc, mybir  # noqa: E402
from concourse.bass_utils import run_bass_kernel_spmd  # noqa: E402

# Problem constants
T, N, V, C = 32, 16, 64, 128
mT, mV = 32, 64
NCORES = 8
SPC = N // NCORES  # scenes per core = 2
Q = V * T          # 2048 queries per scene
QH = 1024          # q processed per half
NEG = -1.0e5       # pad-column bias (exp -> exactly 0.0)

F32 = mybir.dt.float32
BF16 = mybir.dt.bfloat16
FP16 = mybir.dt.float16

TRACE = False            # set by test.py for profiling runs
LAST_RESULT = None       # BassKernelResults of the last run


def _section_cfg(NVP):
    KKP = mT * NVP
    NKT = KKP // 128
    kt_ranges = []
    for kt in range(NKT):
        rngs = []
        p = 0
        while p < 128:
            mtv = (128 * kt + p) // NVP
            p1 = min(128, (mtv + 1) * NVP - 128 * kt)
            rngs.append((p, p1, mtv))
            p = p1
        kt_ranges.append(rngs)
    return dict(NVP=NVP, KKP=KKP, NKT=NKT, ranges=kt_ranges)


def _build_graph(nvps):
    cfgs = [_section_cfg(v) for v in nvps]

    nc = bacc.Bacc("TRN2", target_bir_lowering=False, debug=False,
                   num_devices=NCORES)

    pt = nc.dram_tensor("pt", [SPC, C, Q], FP16, kind="ExternalInput").ap()
    mt_ = [nc.dram_tensor(f"mt{n}", [C, cfgs[n]["KKP"]], FP16,
                          kind="ExternalInput").ap() for n in range(SPC)]
    mb = [nc.dram_tensor(f"mb{n}", [cfgs[n]["NKT"], 128], F32,
                         kind="ExternalInput").ap() for n in range(SPC)]
    g2 = [nc.dram_tensor(f"g2_{n}", [cfgs[n]["NKT"], 128, mT], BF16,
                         kind="ExternalInput").ap() for n in range(SPC)]
    hmat = [nc.dram_tensor(f"hmat{n}", [cfgs[n]["NKT"], 128, mV], BF16,
                           kind="ExternalInput").ap() for n in range(SPC)]
    wqT = nc.dram_tensor("wqT", [C, C], FP16, kind="ExternalInput").ap()
    wkT = nc.dram_tensor("wkT", [C, C], FP16, kind="ExternalInput").ap()
    wvT = nc.dram_tensor("wvT", [C, C], FP16, kind="ExternalInput").ap()
    bq = nc.dram_tensor("bq", [C], F32, kind="ExternalInput").ap()
    bk = nc.dram_tensor("bk", [C], F32, kind="ExternalInput").ap()
    bo = nc.dram_tensor("bo", [C], F32, kind="ExternalInput").ap()  # mT*bv

    out = nc.dram_tensor("out", [SPC, C, Q], F32, kind="ExternalOutput").ap()
    att = nc.dram_tensor("att", [SPC, mV, V], F32, kind="ExternalOutput").ap()
    zdr = nc.dram_tensor("zdr", [SPC, 2, mT, QH], BF16).ap()  # scratch

    from contextlib import ExitStack

    with tile.TileContext(nc) as tc, ExitStack() as ctx:
        consts = ctx.enter_context(tc.tile_pool(name="consts", bufs=1))
        pt_pool = ctx.enter_context(tc.tile_pool(name="pt", bufs=2))
        mt_pool = ctx.enter_context(tc.tile_pool(name="mtp", bufs=2))
        qt_pool = ctx.enter_context(tc.tile_pool(name="qt", bufs=4))
        kt_pool = ctx.enter_context(tc.tile_pool(name="kt", bufs=4))
        val_pool = ctx.enter_context(tc.tile_pool(name="val", bufs=2))
        mb_pool = ctx.enter_context(tc.tile_pool(name="mb", bufs=2))
        h_pool = ctx.enter_context(tc.tile_pool(name="hm", bufs=2))
        a_pool = ctx.enter_context(tc.tile_pool(name="a", bufs=2))
        rz_pool = ctx.enter_context(tc.tile_pool(name="rz", bufs=1))
        rzrep_pool = ctx.enter_context(tc.tile_pool(name="rzrep", bufs=20))
        osb_pool = ctx.enter_context(tc.tile_pool(name="osb", bufs=1))
        asb_pool = ctx.enter_context(tc.tile_pool(name="asb", bufs=2))
        ps1 = ctx.enter_context(tc.tile_pool(name="ps1", bufs=2, space="PSUM"))
        psz = ctx.enter_context(tc.tile_pool(name="psz", bufs=1, space="PSUM"))
        psb = ctx.enter_context(tc.tile_pool(name="psb", bufs=2, space="PSUM"))

        # ---- constants (loaded once) ----
        wq_sb = consts.tile([C, C], FP16)
        nc.sync.dma_start(wq_sb[:], wqT[:])
        wk_sb = consts.tile([C, C], FP16)
        nc.sync.dma_start(wk_sb[:], wkT[:])
        wv_sb = consts.tile([C, C], FP16)
        nc.sync.dma_start(wv_sb[:], wvT[:])
        bq_sb = consts.tile([C, 1], F32)
        nc.sync.dma_start(bq_sb[:], bq[:, None])
        bk_sb = consts.tile([C, 1], F32)
        nc.sync.dma_start(bk_sb[:], bk[:, None])
        bo_sb = consts.tile([C, 1], F32)
        nc.sync.dma_start(bo_sb[:], bo[:, None])
        g2_sb = []
        for n in range(SPC):
            g = consts.tile([128, cfgs[n]["NKT"], mT], BF16, name=f"g2sb{n}")
            nc.sync.dma_start(g[:], g2[n].rearrange("k p m -> p k m"))
            g2_sb.append(g)

        # PE warm-up: dummy matmuls while input DMAs stream, so the HAM
        # un-throttles the PE clock before the first real matmul.
        wsrc = consts.tile([128, 512], FP16)
        nc.vector.memset(wsrc[:], 0.0)
        wps = ps1.tile([128, QH], F32, tag="ps1")
        for i in range(16):
            nc.tensor.matmul(wps[:, (i % 2) * 512:(i % 2 + 1) * 512],
                             wq_sb[:], wsrc[:], start=True, stop=True)
        wsk = consts.tile([128, 512], BF16)
        nc.scalar.activation(wsk[:], wps[:, :512],
                             mybir.ActivationFunctionType.Copy)
        nc.sync.dma_start(zdr[0][0][:, :512], wsk[:mT, :])

        def load_inputs(n):
            KKP = cfgs[n]["KKP"]
            pt_sb = pt_pool.tile([C, Q], FP16)
            for o in range(0, Q, 512):
                nc.sync.dma_start(pt_sb[:, o:o + 512], pt[n][:, o:o + 512])
            mt_sb = mt_pool.tile([C, KKP], FP16, tag="mtp", name=f"mtsb{n}")
            for o in range(0, KKP, 512):
                nc.sync.dma_start(mt_sb[:, o:o + 512], mt_[n][:, o:o + 512])
            mb_sb = mb_pool.tile([128, cfgs[n]["NKT"]], F32, tag="mb",
                                 name=f"mbsb{n}")
            nc.sync.dma_start(mb_sb[:], mb[n].rearrange("k p -> p k"))
            h_sb = h_pool.tile([128, cfgs[n]["NKT"], mV], BF16, tag="hm",
                               name=f"hsb{n}")
            nc.sync.dma_start(h_sb[:], hmat[n].rearrange("k p m -> p k m"))
            return pt_sb, mt_sb, mb_sb, h_sb

        def load_scene(n, inp):
            """Projections for scene n."""
            pt_sb, mt_sb, mb_sb, h_sb = inp
            KKP, NKT = cfgs[n]["KKP"], cfgs[n]["NKT"]

            # Q^T[c_out, q] / K^T[c_out, q] : lhsT = W^T, rhs = Pp^T/Mp^T
            # (separate tiles per region for fine-grained consumer deps)
            qt_sb = []
            for h in range(2):
                qh_sb = qt_pool.tile([C, QH], FP16, tag="qt",
                                     name=f"qt_{n}_{h}")
                pp = ps1.tile([128, QH], F32, tag="ps1")
                for j in range(2):
                    sl = slice(h * QH + j * 512, h * QH + (j + 1) * 512)
                    nc.tensor.matmul(pp[:, j * 512:(j + 1) * 512],
                                     wq_sb[:], pt_sb[:, sl],
                                     start=True, stop=True)
                nc.vector.tensor_scalar(
                    qh_sb[:], pp[:], bq_sb[:], None,
                    mybir.AluOpType.add)
                qt_sb.append(qh_sb)

            kt_sb = []
            ko = 0
            while ko < KKP:
                w = min(QH, KKP - ko)
                kc_sb = kt_pool.tile([C, QH], FP16, tag="kt",
                                     name=f"kt_{n}_{ko}")
                pp = ps1.tile([128, QH], F32, tag="ps1")
                jo = 0
                while jo < w:
                    jw = min(512, w - jo)
                    nc.tensor.matmul(pp[:, jo:jo + jw], wk_sb[:],
                                     mt_sb[:, ko + jo:ko + jo + jw],
                                     start=True, stop=True)
                    jo += jw
                nc.vector.tensor_scalar(
                    kc_sb[:, :w], pp[:, :w], bk_sb[:], None,
                    mybir.AluOpType.add)
                kt_sb.append(kc_sb)
                ko += w

            # Val[k', c] (no bias): lhsT = Mp^T tile, rhs = Wv^T
            val_sb = val_pool.tile([128, NKT, C], BF16, tag="val",
                                   name=f"val{n}")
            ko = 0
            while ko < NKT:
                kw = min(8, NKT - ko)
                pv = ps1.tile([128, QH], F32, tag="ps1")
                for j in range(kw):
                    k = ko + j
                    nc.tensor.matmul(
                        pv[:, j * 128:(j + 1) * 128],
                        mt_sb[:, k * 128:(k + 1) * 128], wv_sb[:],
                        start=True, stop=True)
                nc.vector.tensor_copy(
                    val_sb[:, ko:ko + kw, :].rearrange("p a b -> p (a b)"),
                    pv[:, :kw * 128])
                ko += kw
            return dict(qt=qt_sb, kt=kt_sb, val=val_sb, mb=mb_sb, hm=h_sb)

        def pass_a_gen(n, st, h, box):
            """Scores, exp, Z, reciprocal + replicate, normalize."""
            NKT = cfgs[n]["NKT"]
            a_all = a_pool.tile([128, NKT, QH], BF16, tag=f"a{n}",
                                name=f"a_{n}_{h}")
            box.append(a_all)
            zps = psz.tile([mT, QH], F32, tag="psz")
            for k in range(NKT):
                ss = ps1.tile([128, QH], F32, tag="ps1")
                kt_t = st["kt"][k // 8]
                ksl = slice((k % 8) * 128, (k % 8 + 1) * 128)
                for j in range(2):
                    nc.tensor.matmul(
                        ss[:, j * 512:(j + 1) * 512],
                        kt_t[:, ksl],
                        st["qt"][h][:, j * 512:(j + 1) * 512],
                        start=True, stop=True)
                # A = exp(S^T + padbias)  -> bf16
                nc.scalar.activation(
                    a_all[:, k, :], ss[:],
                    mybir.ActivationFunctionType.Exp,
                    bias=st["mb"][:, k:k + 1])
                # Z[mt, q] += G2_kt^T @ A_kt
                for j in range(2):
                    nc.tensor.matmul(
                        zps[:, j * 512:(j + 1) * 512],
                        g2_sb[n][:, k, :],
                        a_all[:, k, j * 512:(j + 1) * 512],
                        start=(k == 0), stop=(k == NKT - 1))
                yield

            # reciprocal of Z, bounce through DRAM to replicate
            rz_sb = rz_pool.tile([mT, QH], F32, tag="rzf")
            nc.vector.reciprocal_approx_fast(rz_sb[:], zps[:])
            rzb_sb = rz_pool.tile([mT, QH], BF16, tag="rzb")
            nc.vector.tensor_copy(rzb_sb[:], rz_sb[:])
            nc.gpsimd.dma_start(zdr[n][h], rzb_sb[:])
            rzrep = []
            for k in range(NKT):
                rzk = rzrep_pool.tile([128, QH], BF16, tag="rzrep")
                for (p0, p1, mtv) in cfgs[n]["ranges"][k]:
                    nc.gpsimd.dma_start(
                        rzk[p0:p1, :],
                        zdr[n][h][mtv][None, :].to_broadcast((p1 - p0, QH)))
                rzrep.append(rzk)
            yield
            # normalize in place (DVE overlaps the next unit's pass A)
            for k in range(NKT):
                nc.vector.tensor_mul(
                    a_all[:, k, :], a_all[:, k, :], rzrep[k][:])
                if k % 3 == 2:
                    yield

        def pass_b_gen(n, st, h, a_all):
            """out^T/att^T accumulation + eviction (2 MMs per weight load)."""
            NKT = a_all.shape[1]
            po = [psb.tile([C, 512], F32, tag="psb", name=f"po{j}")
                  for j in range(2)]
            for k in range(NKT):
                for j in range(2):
                    nc.tensor.matmul(
                        po[j][:], st["val"][:, k, :],
                        a_all[:, k, j * 512:(j + 1) * 512],
                        start=(k == 0), stop=(k == NKT - 1))
                if k % 2:
                    yield
            for j in range(2):
                qsl = slice(h * QH + j * 512, h * QH + (j + 1) * 512)
                o_sb = osb_pool.tile([C, 512], F32)
                nc.scalar.activation(
                    o_sb[:], po[j][:],
                    mybir.ActivationFunctionType.Identity, bias=bo_sb[:])
                nc.sync.dma_start(out[n][:, qsl], o_sb[:])
            yield
            pa = [ps1.tile([mV, 512], F32, tag="ps1", name=f"pa{j}")
                  for j in range(2)]
            for k in range(NKT):
                for j in range(2):
                    nc.tensor.matmul(
                        pa[j][:], st["hm"][:, k, :],
                        a_all[:, k, j * 512:(j + 1) * 512],
                        start=(k == 0), stop=(k == NKT - 1))
                if k % 2:
                    yield
            for j in range(2):
                at_sb = asb_pool.tile([mV, 512 // T], F32)
                nc.vector.tensor_reduce(
                    at_sb[:], pa[j][:].rearrange("p (v t) -> p v t", t=T),
                    axis=mybir.AxisListType.X, op=mybir.AluOpType.add)
                qv = (h * QH + j * 512) // T
                nc.sync.dma_start(att[n][:, qv:qv + 512 // T], at_sb[:])

        def drive(*gens):
            gens = [g for g in gens if g is not None]
            while gens:
                alive = []
                for g in gens:
                    try:
                        next(g)
                        alive.append(g)
                    except StopIteration:
                        pass
                gens = alive

        inputs = [load_inputs(n) for n in range(SPC)]
        sts = [load_scene(0, inputs[0])]
        units = [(n, h) for n in range(SPC) for h in range(2)]
        boxes = {u: [] for u in units}
        # phase 1: all pass-A units (recip/broadcast/normalize chains hide
        # under the next unit's scores); phase 2: dense pass-B matmul block
        for u in units:
            n, h = u
            if h == 0 and n > 0:
                sts.append(load_scene(n, inputs[n]))
            drive(pass_a_gen(n, sts[n], h, boxes[u]))
        # stagger B units: unit u's att phase (ps1 slots) overlaps unit
        # u+1's AV phase (psb slots)
        b_prev = None
        for u in units:
            n, h = u
            gb = pass_b_gen(n, sts[n], h, boxes[u][0])
            av_yields = cfgs[n]["NKT"] // 2 + 1
            for _ in range(av_yields):
                next(gb, None)
                if b_prev is not None:
                    next(b_prev, None)
            if b_prev is not None:
                drive(b_prev)
            b_prev = gb
        drive(b_prev)

    nc.compile()
    return nc


_NC = None
_NVPS = None


def _get_nc(nvps):
    global _NC, _NVPS
    if _NC is None or _NVPS != nvps:
        _NC = _build_graph(nvps)
        _NVPS = nvps
    return _NC


def _scene_data(m4n, maskn, NVP):
    """Compacted Mp^T, pad bias, att selector for one scene at width NVP."""
    KKP = mT * NVP
    NKT = KKP // 128
    idx = np.nonzero(maskn)[0]
    nv = len(idx)
    assert nv <= NVP
    mpt = np.zeros((C, mT, NVP), dtype=np.float16)
    mpt[:, :, :nv] = m4n[:, :, idx]
    mbias = np.full((NKT, 128), NEG, dtype=np.float32)
    hm = np.zeros((NKT, 128, mV), dtype=ml_dtypes.bfloat16)
    hscale = np.float32(1.0 / (T * mT))
    for kt in range(NKT):
        for p in range(128):
            c = (128 * kt + p) % NVP
            if c < nv:
                mbias[kt, p] = 0.0
                hm[kt, p, idx[c]] = hscale
    return np.ascontiguousarray(mpt.reshape(C, KKP)), mbias, hm


def _g2_structural(NVP):
    KKP = mT * NVP
    NKT = KKP // 128
    g2 = np.zeros((NKT, 128, mT), dtype=ml_dtypes.bfloat16)
    for kt in range(NKT):
        for p in range(128):
            g2[kt, p, (128 * kt + p) // NVP] = 1.0
    return g2


def kernel(P, M, mask, Wq, bq, Wk, bk, Wv, bv, V=64, mV=64, **_ignored):
    global LAST_RESULT
    assert int(V) == 64 and int(mV) == 64
    mask2 = np.asarray(mask).astype(bool)[:, 0, :, 0]   # [N, mV]
    nv = mask2.sum(axis=1)
    # sort scenes by valid count; section s takes scenes perm[s*8 .. s*8+7]
    perm = np.argsort(-nv, kind="stable")
    nvps = []
    for s in range(SPC):
        mx = int(nv[perm[s * NCORES:(s + 1) * NCORES]].max())
        nvps.append(int(min(64, max(16, ((mx + 3) // 4) * 4))))
    nvps = tuple(nvps)
    nc = _get_nc(nvps)

    P = np.asarray(P, dtype=np.float32)
    M = np.asarray(M, dtype=np.float32)
    ppt = np.transpose(P.reshape(T, N, V, C), (1, 3, 2, 0)).reshape(N, C, Q)
    m4 = np.transpose(M.reshape(mT, N, mV, C), (1, 3, 0, 2))  # [N, C, mT, mV]

    wqT = np.ascontiguousarray(np.asarray(Wq, np.float32).T).astype(np.float16)
    wkT = np.ascontiguousarray(np.asarray(Wk, np.float32).T).astype(np.float16)
    wvT = np.ascontiguousarray(np.asarray(Wv, np.float32).T).astype(np.float16)
    bo = (float(mT) * np.asarray(bv, np.float32)).astype(np.float32)
    g2s = [_g2_structural(nvps[s]) for s in range(SPC)]

    in_maps = []
    for i in range(NCORES):
        im = {
            "wqT": wqT, "wkT": wkT, "wvT": wvT,
            "bq": np.asarray(bq, np.float32),
            "bk": np.asarray(bk, np.float32),
            "bo": bo,
        }
        pts = []
        for s in range(SPC):
            sc = int(perm[s * NCORES + i])
            mpt, mbias, hm = _scene_data(m4[sc], mask2[sc], nvps[s])
            pts.append(ppt[sc].astype(np.float16))
            im[f"mt{s}"] = mpt
            im[f"mb{s}"] = mbias
            im[f"hmat{s}"] = np.ascontiguousarray(hm)
            im[f"g2_{s}"] = g2s[s]
        im["pt"] = np.ascontiguousarray(np.stack(pts))
        in_maps.append(im)

    res = run_bass_kernel_spmd(
        nc, in_maps, core_ids=list(range(NCORES)), trace=TRACE,
    )
    LAST_RESULT = res

    full_out = np.empty((T, N * V, C), dtype=np.float32)
    full_att = np.empty((N, V, mV), dtype=np.float32)
    for s in range(SPC):
        for i in range(NCORES):
            sc = int(perm[s * NCORES + i])
            o = res.results[i]["out"][s]          # [C, Q], q=(v,t)
            a = res.results[i]["att"][s]          # [mV, V]
            full_out[:, sc * V:(sc + 1) * V, :] = np.transpose(
                o.reshape(C, V, T), (2, 1, 0))
            full_att[sc] = a.T
    return np.ascontiguousarray(full_out), np.ascontiguousarray(full_att)


# revision 40
# speedup vs baseline: 1.0467x; 1.0467x over previous
"""Trainium2 Bass kernel for nn_Attention_51608327028778 (sparse_attention).

Problem (hardcoded shapes):
  T=32, N=16, V=64, C=128, mT=32, mV=64
  P:[32,1024,128] M:[32,1024,128] mask:[16,1,64,1] Wq/Wk/Wv:[128,128] b*:[128]
  out:[32,1024,128], att:[16,64,64]

Key ideas:
 - the mask is per-(scene, mv) and shared across mT, so the mv axis is
   COMPACTED host-side to the valid entries (padded to a per-section NVP),
   shrinking scores / softmax / AV / att work by ~ NVP/mV.  Keys are ordered
   k' = mt*NVP + c (c = compact mv index) so softmax-over-mv becomes
   partition-block sums.
 - scenes are SORTED by valid count and paired big-with-big across cores:
   graph section 0 (first scene on every core) gets the 8 largest scenes,
   section 1 the 8 smallest, so each section pads to its own (smaller) NVP.
 - schedule: PE warm-up burst; then all softmax units (scores/exp/Z/recip/
   normalize, chains hidden under the next unit's scores); then one dense
   all-matmul output block.

Per-scene math (scene n):
  Q = Pp@Wq.T+bq ; K = Mpc@Wk.T+bk ; Val = Mpc@Wv.T (+bv folded into output)
  S^T[k',q] = K@Q^T                    (fp16 matmuls, 1 cyc/row)
  A = exp(S^T + padbias[k'])           (ACT, per-partition bias, bf16 out)
  Z[mt,q] += G2_kt^T @ A_kt            (TensorE; G2 structural)
  rz = reciprocal_approx_fast(Z); replicated across partitions via DRAM
  a^T = A * rz_rep                     (DVE bf16 2x)
  out^T[c,q] += Val_kt^T a^T           (TensorE, accum over k' tiles)
  att^T[mv,q] += (H_n/1024)^T a^T      (TensorE), then reduce over t (DVE)
  out^T[c,q] += mT*bv[c]               (ACT bias on PSUM eviction; exact
                                        because softmax sums to 1 per (q,mt))

Sharding: data-parallel over scenes, 2 scenes per core, 8 cores, full I/O
resharded host-side.
"""

import os
import sys

import numpy as np

for _p in ("/opt/trn_rl_repo",):
    if _p not in sys.path and os.path.isdir(_p):
        sys.path.append(_p)

import ml_dtypes  # noqa: E402

import concourse.tile as tile  # noqa: E402
from concourse import bacc, mybir  # noqa: E402
from concourse.bass_utils import run_bass_kernel_spmd  # noqa: E402

# Problem constants
T, N, V, C = 32, 16, 64, 128
mT, mV = 32, 64
NCORES = 8
SPC = N // NCORES  # scenes per core = 2
Q = V * T          # 2048 queries per scene
QH = 1024          # q processed per half
NEG = -1.0e5       # pad-column bias (exp -> exactly 0.0)

F32 = mybir.dt.float32
BF16 = mybir.dt.bfloat16
FP16 = mybir.dt.float16

TRACE = False            # set by test.py for profiling runs
LAST_RESULT = None       # BassKernelResults of the last run


def _section_cfg(NVP):
    KKP = mT * NVP
    NKT = KKP // 128
    kt_ranges = []
    for kt in range(NKT):
        rngs = []
        p = 0
        while p < 128:
            mtv = (128 * kt + p) // NVP
            p1 = min(128, (mtv + 1) * NVP - 128 * kt)
            rngs.append((p, p1, mtv))
            p = p1
        kt_ranges.append(rngs)
    return dict(NVP=NVP, KKP=KKP, NKT=NKT, ranges=kt_ranges)


def _build_graph(nvps):
    cfgs = [_section_cfg(v) for v in nvps]

    nc = bacc.Bacc("TRN2", target_bir_lowering=False, debug=False,
                   num_devices=NCORES)

    pt = nc.dram_tensor("pt", [SPC, C, Q], FP16, kind="ExternalInput").ap()
    mt_ = [nc.dram_tensor(f"mt{n}", [C, cfgs[n]["KKP"]], FP16,
                          kind="ExternalInput").ap() for n in range(SPC)]
    mb = [nc.dram_tensor(f"mb{n}", [cfgs[n]["NKT"], 128], F32,
                         kind="ExternalInput").ap() for n in range(SPC)]
    g2 = [nc.dram_tensor(f"g2_{n}", [cfgs[n]["NKT"], 128, mT], BF16,
                         kind="ExternalInput").ap() for n in range(SPC)]
    hmat = [nc.dram_tensor(f"hmat{n}", [cfgs[n]["NKT"], 128, mV], BF16,
                           kind="ExternalInput").ap() for n in range(SPC)]
    wqT = nc.dram_tensor("wqT", [C, C], FP16, kind="ExternalInput").ap()
    wkT = nc.dram_tensor("wkT", [C, C], FP16, kind="ExternalInput").ap()
    wvT = nc.dram_tensor("wvT", [C, C], FP16, kind="ExternalInput").ap()
    bq = nc.dram_tensor("bq", [C], F32, kind="ExternalInput").ap()
    bk = nc.dram_tensor("bk", [C], F32, kind="ExternalInput").ap()
    bo = nc.dram_tensor("bo", [C], F32, kind="ExternalInput").ap()  # mT*bv

    out = nc.dram_tensor("out", [SPC, C, Q], F32, kind="ExternalOutput").ap()
    att = nc.dram_tensor("att", [SPC, mV, V], F32, kind="ExternalOutput").ap()
    zdr = nc.dram_tensor("zdr", [SPC, 2, mT, QH], BF16).ap()  # scratch

    from contextlib import ExitStack

    with tile.TileContext(nc) as tc, ExitStack() as ctx:
        consts = ctx.enter_context(tc.tile_pool(name="consts", bufs=1))
        pt_pool = ctx.enter_context(tc.tile_pool(name="pt", bufs=2))
        mt_pool = ctx.enter_context(tc.tile_pool(name="mtp", bufs=2))
        qt_pool = ctx.enter_context(tc.tile_pool(name="qt", bufs=4))
        kt_pool = ctx.enter_context(tc.tile_pool(name="kt", bufs=4))
        val_pool = ctx.enter_context(tc.tile_pool(name="val", bufs=2))
        mb_pool = ctx.enter_context(tc.tile_pool(name="mb", bufs=2))
        h_pool = ctx.enter_context(tc.tile_pool(name="hm", bufs=2))
        a_pool = ctx.enter_context(tc.tile_pool(name="a", bufs=2))
        rz_pool = ctx.enter_context(tc.tile_pool(name="rz", bufs=1))
        rzrep_pool = ctx.enter_context(tc.tile_pool(name="rzrep", bufs=20))
        osb_pool = ctx.enter_context(tc.tile_pool(name="osb", bufs=1))
        asb_pool = ctx.enter_context(tc.tile_pool(name="asb", bufs=2))
        ps1 = ctx.enter_context(tc.tile_pool(name="ps1", bufs=2, space="PSUM"))
        psz = ctx.enter_context(tc.tile_pool(name="psz", bufs=1, space="PSUM"))
        psb = ctx.enter_context(tc.tile_pool(name="psb", bufs=2, space="PSUM"))

        # ---- constants (loaded once) ----
        wq_sb = consts.tile([C, C], FP16)
        nc.sync.dma_start(wq_sb[:], wqT[:])
        wk_sb = consts.tile([C, C], FP16)
        nc.sync.dma_start(wk_sb[:], wkT[:])
        wv_sb = consts.tile([C, C], FP16)
        nc.sync.dma_start(wv_sb[:], wvT[:])
        bq_sb = consts.tile([C, 1], F32)
        nc.sync.dma_start(bq_sb[:], bq[:, None])
        bk_sb = consts.tile([C, 1], F32)
        nc.sync.dma_start(bk_sb[:], bk[:, None])
        bo_sb = consts.tile([C, 1], F32)
        nc.sync.dma_start(bo_sb[:], bo[:, None])
        g2_sb = []
        for n in range(SPC):
            g = consts.tile([128, cfgs[n]["NKT"], mT], BF16, name=f"g2sb{n}")
            nc.sync.dma_start(g[:], g2[n].rearrange("k p m -> p k m"))
            g2_sb.append(g)

        # PE warm-up: dummy matmuls while input DMAs stream, so the HAM
        # un-throttles the PE clock before the first real matmul.
        wsrc = consts.tile([128, 512], FP16)
        nc.vector.memset(wsrc[:], 0.0)
        wps = ps1.tile([128, QH], F32, tag="ps1")
        for i in range(16):
            nc.tensor.matmul(wps[:, (i % 2) * 512:(i % 2 + 1) * 512],
                             wq_sb[:], wsrc[:], start=True, stop=True)
        wsk = consts.tile([128, 512], BF16)
        nc.scalar.activation(wsk[:], wps[:, :512],
                             mybir.ActivationFunctionType.Copy)
        nc.sync.dma_start(zdr[0][0][:, :512], wsk[:mT, :])

        def load_inputs(n):
            KKP = cfgs[n]["KKP"]
            pt_sb = pt_pool.tile([C, Q], FP16)
            for o in range(0, Q, 512):
                nc.sync.dma_start(pt_sb[:, o:o + 512], pt[n][:, o:o + 512])
            mt_sb = mt_pool.tile([C, KKP], FP16, tag="mtp", name=f"mtsb{n}")
            for o in range(0, KKP, 512):
                nc.sync.dma_start(mt_sb[:, o:o + 512], mt_[n][:, o:o + 512])
            mb_sb = mb_pool.tile([128, cfgs[n]["NKT"]], F32, tag="mb",
                                 name=f"mbsb{n}")
            nc.sync.dma_start(mb_sb[:], mb[n].rearrange("k p -> p k"))
            h_sb = h_pool.tile([128, cfgs[n]["NKT"], mV], BF16, tag="hm",
                               name=f"hsb{n}")
            nc.sync.dma_start(h_sb[:], hmat[n].rearrange("k p m -> p k m"))
            return pt_sb, mt_sb, mb_sb, h_sb

        def load_scene(n, inp):
            """Projections for scene n."""
            pt_sb, mt_sb, mb_sb, h_sb = inp
            KKP, NKT = cfgs[n]["KKP"], cfgs[n]["NKT"]

            # Q^T[c_out, q] / K^T[c_out, q] : lhsT = W^T, rhs = Pp^T/Mp^T
            # (separate tiles per region for fine-grained consumer deps)
            qt_sb = []
            for h in range(2):
                qh_sb = qt_pool.tile([C, QH], FP16, tag="qt",
                                     name=f"qt_{n}_{h}")
                pp = ps1.tile([128, QH], F32, tag="ps1")
                for j in range(2):
                    sl = slice(h * QH + j * 512, h * QH + (j + 1) * 512)
                    nc.tensor.matmul(pp[:, j * 512:(j + 1) * 512],
                                     wq_sb[:], pt_sb[:, sl],
                                     start=True, stop=True)
                nc.vector.tensor_scalar(
                    qh_sb[:], pp[:], bq_sb[:], None,
                    mybir.AluOpType.add)
                qt_sb.append(qh_sb)

            kt_sb = []
            ko = 0
            while ko < KKP:
                w = min(QH, KKP - ko)
                kc_sb = kt_pool.tile([C, QH], FP16, tag="kt",
                                     name=f"kt_{n}_{ko}")
                pp = ps1.tile([128, QH], F32, tag="ps1")
                jo = 0
                while jo < w:
                    jw = min(512, w - jo)
                    nc.tensor.matmul(pp[:, jo:jo + jw], wk_sb[:],
                                     mt_sb[:, ko + jo:ko + jo + jw],
                                     start=True, stop=True)
                    jo += jw
                nc.vector.tensor_scalar(
                    kc_sb[:, :w], pp[:, :w], bk_sb[:], None,
                    mybir.AluOpType.add)
                kt_sb.append(kc_sb)
                ko += w

            # Val[k', c] (no bias): lhsT = Mp^T tile, rhs = Wv^T
            val_sb = val_pool.tile([128, NKT, C], BF16, tag="val",
                                   name=f"val{n}")
            ko = 0
            while ko < NKT:
                kw = min(8, NKT - ko)
                pv = ps1.tile([128, QH], F32, tag="ps1")
                for j in range(kw):
                    k = ko + j
                    nc.tensor.matmul(
                        pv[:, j * 128:(j + 1) * 128],
                        mt_sb[:, k * 128:(k + 1) * 128], wv_sb[:],
                        start=True, stop=True)
                nc.vector.tensor_copy(
                    val_sb[:, ko:ko + kw, :].rearrange("p a b -> p (a b)"),
                    pv[:, :kw * 128])
                ko += kw
            return dict(qt=qt_sb, kt=kt_sb, val=val_sb, mb=mb_sb, hm=h_sb)

        def pass_a_gen(n, st, h, box):
            """Scores, exp, Z, reciprocal + replicate, normalize."""
            NKT = cfgs[n]["NKT"]
            a_all = a_pool.tile([128, NKT, QH], BF16, tag=f"a{n}",
                                name=f"a_{n}_{h}")
            box.append(a_all)
            zps = psz.tile([mT, QH], F32, tag="psz")
            for k in range(NKT):
                ss = ps1.tile([128, QH], F32, tag="ps1")
                kt_t = st["kt"][k // 8]
                ksl = slice((k % 8) * 128, (k % 8 + 1) * 128)
                for j in range(2):
                    nc.tensor.matmul(
                        ss[:, j * 512:(j + 1) * 512],
                        kt_t[:, ksl],
                        st["qt"][h][:, j * 512:(j + 1) * 512],
                        start=True, stop=True)
                # A = exp(S^T + padbias)  -> bf16
                nc.scalar.activation(
                    a_all[:, k, :], ss[:],
                    mybir.ActivationFunctionType.Exp,
                    bias=st["mb"][:, k:k + 1])
                # Z[mt, q] += G2_kt^T @ A_kt
                for j in range(2):
                    nc.tensor.matmul(
                        zps[:, j * 512:(j + 1) * 512],
                        g2_sb[n][:, k, :],
                        a_all[:, k, j * 512:(j + 1) * 512],
                        start=(k == 0), stop=(k == NKT - 1))
                yield

            # reciprocal of Z, bounce through DRAM to replicate
            rz_sb = rz_pool.tile([mT, QH], F32, tag="rzf")
            nc.vector.reciprocal_approx_fast(rz_sb[:], zps[:])
            rzb_sb = rz_pool.tile([mT, QH], BF16, tag="rzb")
            nc.vector.tensor_copy(rzb_sb[:], rz_sb[:])
            nc.gpsimd.dma_start(zdr[n][h], rzb_sb[:])
            rzrep = []
            for k in range(NKT):
                rzk = rzrep_pool.tile([128, QH], BF16, tag="rzrep")
                for (p0, p1, mtv) in cfgs[n]["ranges"][k]:
                    nc.gpsimd.dma_start(
                        rzk[p0:p1, :],
                        zdr[n][h][mtv][None, :].to_broadcast((p1 - p0, QH)))
                rzrep.append(rzk)
            yield
            # normalize in place (DVE overlaps the next unit's pass A)
            for k in range(NKT):
                nc.vector.tensor_mul(
                    a_all[:, k, :], a_all[:, k, :], rzrep[k][:])
                if k % 3 == 2:
                    yield

        def pass_b_gen(n, st, h, a_all):
            """out^T/att^T accumulation + eviction (2 MMs per weight load)."""
            NKT = a_all.shape[1]
            po = [psb.tile([C, 512], F32, tag="psb", name=f"po{j}")
                  for j in range(2)]
            for k in range(NKT):
                for j in range(2):
                    nc.tensor.matmul(
                        po[j][:], st["val"][:, k, :],
                        a_all[:, k, j * 512:(j + 1) * 512],
                        start=(k == 0), stop=(k == NKT - 1))
                if k % 2:
                    yield
            for j in range(2):
                qsl = slice(h * QH + j * 512, h * QH + (j + 1) * 512)
                o_sb = osb_pool.tile([C, 512], F32)
                nc.scalar.activation(
                    o_sb[:], po[j][:],
                    mybir.ActivationFunctionType.Identity, bias=bo_sb[:])
                nc.sync.dma_start(out[n][:, qsl], o_sb[:])
            yield
            pa = [ps1.tile([mV, 512], F32, tag="ps1", name=f"pa{j}")
                  for j in range(2)]
            for k in range(NKT):
                for j in range(2):
                    nc.tensor.matmul(
                        pa[j][:], st["hm"][:, k, :],
                        a_all[:, k, j * 512:(j + 1) * 512],
                        start=(k == 0), stop=(k == NKT - 1))
                if k % 2:
                    yield
            for j in range(2):
                at_sb = asb_pool.tile([mV, 512 // T], F32)
                nc.vector.tensor_reduce(
                    at_sb[:], pa[j][:].rearrange("p (v t) -> p v t", t=T),
                    axis=mybir.AxisListType.X, op=mybir.AluOpType.add)
                qv = (h * QH + j * 512) // T
                nc.sync.dma_start(att[n][:, qv:qv + 512 // T], at_sb[:])

        def drive(*gens):
            gens = [g for g in gens if g is not None]
            while gens:
                alive = []
                for g in gens:
                    try:
                        next(g)
                        alive.append(g)
                    except StopIteration:
                        pass
                gens = alive

        inputs = [load_inputs(n) for n in range(SPC)]
        sts = [load_scene(0, inputs[0])]
        units = [(n, h) for n in range(SPC) for h in range(2)]
        boxes = {u: [] for u in units}
        # phase 1: all pass-A units (recip/broadcast/normalize chains hide
        # under the next unit's scores); phase 2: dense pass-B matmul block
        for u in units:
            n, h = u
            if h == 0 and n > 0:
                sts.append(load_scene(n, inputs[n]))
            drive(pass_a_gen(n, sts[n], h, boxes[u]))
        for u in units:
            n, h = u
            drive(pass_b_gen(n, sts[n], h, boxes[u][0]))

    nc.compile()
    return nc


_NC = None
_NVPS = None


def _get_nc(nvps):
    global _NC, _NVPS
    if _NC is None or _NVPS != nvps:
        _NC = _build_graph(nvps)
        _NVPS = nvps
    return _NC


def _scene_data(m4n, maskn, NVP):
    """Compacted Mp^T, pad bias, att selector for one scene at width NVP."""
    KKP = mT * NVP
    NKT = KKP // 128
    idx = np.nonzero(maskn)[0]
    nv = len(idx)
    assert nv <= NVP
    mpt = np.zeros((C, mT, NVP), dtype=np.float16)
    mpt[:, :, :nv] = m4n[:, :, idx]
    mbias = np.full((NKT, 128), NEG, dtype=np.float32)
    hm = np.zeros((NKT, 128, mV), dtype=ml_dtypes.bfloat16)
    hscale = np.float32(1.0 / (T * mT))
    for kt in range(NKT):
        for p in range(128):
            c = (128 * kt + p) % NVP
            if c < nv:
                mbias[kt, p] = 0.0
                hm[kt, p, idx[c]] = hscale
    return np.ascontiguousarray(mpt.reshape(C, KKP)), mbias, hm


def _g2_structural(NVP):
    KKP = mT * NVP
    NKT = KKP // 128
    g2 = np.zeros((NKT, 128, mT), dtype=ml_dtypes.bfloat16)
    for kt in range(NKT):
        for p in range(128):
            g2[kt, p, (128 * kt + p) // NVP] = 1.0
    return g2


def kernel(P, M, mask, Wq, bq, Wk, bk, Wv, bv, V=64, mV=64, **_ignored):
    global LAST_RESULT
    assert int(V) == 64 and int(mV) == 64
    mask2 = np.asarray(mask).astype(bool)[:, 0, :, 0]   # [N, mV]
    nv = mask2.sum(axis=1)
    # sort scenes by valid count; section s takes scenes perm[s*8 .. s*8+7]
    perm = np.argsort(-nv, kind="stable")
    nvps = []
    for s in range(SPC):
        mx = int(nv[perm[s * NCORES:(s + 1) * NCORES]].max())
        nvps.append(int(min(64, max(16, ((mx + 3) // 4) * 4))))
    nvps = tuple(nvps)
    nc = _get_nc(nvps)

    P = np.asarray(P, dtype=np.float32)
    M = np.asarray(M, dtype=np.float32)
    ppt = np.transpose(P.reshape(T, N, V, C), (1, 3, 2, 0)).reshape(N, C, Q)
    m4 = np.transpose(M.reshape(mT, N, mV, C), (1, 3, 0, 2))  # [N, C, mT, mV]

    wqT = np.ascontiguousarray(np.asarray(Wq, np.float32).T).astype(np.float16)
    wkT = np.ascontiguousarray(np.asarray(Wk, np.float32).T).astype(np.float16)
    wvT = np.ascontiguousarray(np.asarray(Wv, np.float32).T).astype(np.float16)
    bo = (float(mT) * np.asarray(bv, np.float32)).astype(np.float32)
    g2s = [_g2_structural(nvps[s]) for s in range(SPC)]

    in_maps = []
    for i in range(NCORES):
        im = {
            "wqT": wqT, "wkT": wkT, "wvT": wvT,
            "bq": np.asarray(bq, np.float32),
            "bk": np.asarray(bk, np.float32),
            "bo": bo,
        }
        pts = []
        for s in range(SPC):
            sc = int(perm[s * NCORES + i])
            mpt, mbias, hm = _scene_data(m4[sc], mask2[sc], nvps[s])
            pts.append(ppt[sc].astype(np.float16))
            im[f"mt{s}"] = mpt
            im[f"mb{s}"] = mbias
            im[f"hmat{s}"] = np.ascontiguousarray(hm)
            im[f"g2_{s}"] = g2s[s]
        im["pt"] = np.ascontiguousarray(np.stack(pts))
        in_maps.append(im)

    res = run_bass_kernel_spmd(
        nc, in_maps, core_ids=list(range(NCORES)), trace=TRACE,
    )
    LAST_RESULT = res

    full_out = np.empty((T, N * V, C), dtype=np.float32)
    full_att = np.empty((N, V, mV), dtype=np.float32)
    for s in range(SPC):
        for i in range(NCORES):
            sc = int(perm[s * NCORES + i])
            o = res.results[i]["out"][s]          # [C, Q], q=(v,t)
            a = res.results[i]["att"][s]          # [mV, V]
            full_out[:, sc * V:(sc + 1) * V, :] = np.transpose(
                o.reshape(C, V, T), (2, 1, 0))
            full_att[sc] = a.T
    return np.ascontiguousarray(full_out), np.ascontiguousarray(full_att)
